# revision 1
# baseline (speedup 1.0000x reference)
"""Multi-head causal self-attention with RoPE on 8 Trainium2 NeuronCores.

Reference computation (B=2, S=2048, D=2048, H=16, DH=128):
    xs = hidden_q / sqrt(D)
    q,k,v = xs @ {Wq,Wk,Wv}.T        (reshaped to [B,H,S,DH])
    q,k <- RoPE(q,k)
    scores = q @ k.T / sqrt(DH)  (causal masked)
    p = softmax(scores); attn = p @ v
    out = (attn / sqrt(H*DH)) @ Wo.T

Sharding: 8 cores = 2 (batch) x 4 (head-groups of 4 heads).  Each core
computes its head-group's projections, attention and a partial output
projection; the host sums the 4 partials per batch.

All matmuls run in float32r (TF32-like, full PE rate at N=512).
Layouts on device (per core):
    xT   [D, S]    feature-major activations (host pre-transposed, pre-scaled)
    wqT  [D, 512]  per-group Wq slice, transposed
    scoresT [keys, queries] so softmax-denominators come from a ones-matmul
    attnT [dh, q] accumulated per head, normalized with broadcast reciprocal
    y    [S, D]    natural layout partial output (host sums over groups)
"""

import math
from contextlib import ExitStack

import numpy as np

import concourse.bass as bass
import concourse.mybir as mybir
import concourse.tile as tile
from concourse import bacc
from concourse.bass import ts
from concourse.bass_utils import run_bass_kernel_spmd
from concourse.masks import make_identity

B, S, D, H, DH = 2, 2048, 2048, 16, 128
BASE = 10000.0
G = 4              # head-groups (cores per batch)
HG = H // G        # heads per group = 4
F = HG * DH        # features per group = 512
NT = S // 128      # 16 token tiles
NQB = S // 512     # 4 query blocks
F32 = mybir.dt.float32
F32R = mybir.dt.float32r

_cache = {}


def _rope_tables():
    inv_freq = 1.0 / (BASE ** (np.arange(0, DH, 2, dtype=np.float64) / DH))
    t = np.arange(S, dtype=np.float64)
    freqs = np.outer(t, inv_freq)                       # [S, 64]
    return (np.cos(freqs).astype(np.float32), np.sin(freqs).astype(np.float32))


def _mask_tiles():
    # mask[o][j, q] = 1 if q >= j + 128*o else 0  (diagonal-band tiles)
    o = np.arange(4)[:, None, None]
    j = np.arange(128)[None, :, None]
    q = np.arange(512)[None, None, :]
    return (q >= j + 128 * o).astype(np.float32)        # [4, 128, 512]


def _build(reps=1):
    key = ("nc", reps)
    if key in _cache:
        return _cache[key]
    nc = bacc.Bacc("TRN2", target_bir_lowering=False, debug=False, num_devices=8)

    xT = nc.dram_tensor("xT", [D, S], F32R, kind="ExternalInput")
    wqT = nc.dram_tensor("wqT", [D, F], F32R, kind="ExternalInput")
    wkT = nc.dram_tensor("wkT", [D, F], F32R, kind="ExternalInput")
    wvT = nc.dram_tensor("wvT", [D, F], F32R, kind="ExternalInput")
    woT = nc.dram_tensor("woT", [F, D], F32R, kind="ExternalInput")
    cos_d = nc.dram_tensor("cos", [S, 64], F32R, kind="ExternalInput")
    sin_d = nc.dram_tensor("sin", [S, 64], F32R, kind="ExternalInput")
    msk_d = nc.dram_tensor("masks", [4, 128, 512], F32R, kind="ExternalInput")
    y = nc.dram_tensor("y", [S, D], F32, kind="ExternalOutput")

    # chunked spill tensors (one per 4-token-tile group) so phase-B reloads
    # depend only on their own chunk's spills, not the whole phase A
    q_spill = [nc.dram_tensor(f"q_spill{c}", [F, 512], F32R) for c in range(4)]
    k_spill = [nc.dram_tensor(f"k_spill{c}", [F, 512], F32R) for c in range(4)]
    q_spill_r = [t.ap().rearrange("(hb p) s -> p hb s", p=128) for t in q_spill]
    k_spill_r = [t.ap().rearrange("(hb p) s -> p hb s", p=128) for t in k_spill]

    xT_r = xT.ap().rearrange("(kt p) s -> p kt s", p=128)       # [128, 16, S]

    with tile.TileContext(nc) as tc, ExitStack() as ctx:
        const = ctx.enter_context(tc.tile_pool(name="const", bufs=1))
        vpool = ctx.enter_context(tc.tile_pool(name="vpool", bufs=1))
        ps512 = ctx.enter_context(tc.tile_pool(name="ps512", bufs=6, space="PSUM"))
        # transposes (phase A) and denominators (phase B) share slots
        ps_sm = ctx.enter_context(tc.tile_pool(name="ps_sm", bufs=2, space="PSUM"))

        ones_f = const.tile([128, 1], F32, tag="ones_f")
        nc.gpsimd.memset(ones_f[:], 1.0)
        ones = const.tile([128, 1], F32R, tag="ones")
        nc.vector.tensor_copy(ones[:], ones_f[:])
        ident_f = const.tile([128, 128], F32, tag="ident_f")
        make_identity(nc, ident_f[:])
        ident = const.tile([128, 128], F32R, tag="ident")
        nc.vector.tensor_copy(ident[:], ident_f[:])

        for _rep in range(reps):
            vh_cb = [vpool.tile([128, 4, F], F32R, tag=f"vh{c}", name=f"vh{c}") for c in range(4)]

            # ---------- Phase A: projections + RoPE + transpose + spill ----
            with ExitStack() as actx:
                wpool = actx.enter_context(tc.tile_pool(name="wpool", bufs=1))
                xpool = actx.enter_context(tc.tile_pool(name="xpool", bufs=3))
                rot_pool = actx.enter_context(tc.tile_pool(name="rot", bufs=2))
                tmp_pool = actx.enter_context(tc.tile_pool(name="tmp", bufs=4))
                stage = actx.enter_context(tc.tile_pool(name="stage", bufs=3))

                wq_sb = wpool.tile([128, NT, F], F32R, tag="wq")
                wk_sb = wpool.tile([128, NT, F], F32R, tag="wk")
                wv_sb = wpool.tile([128, NT, F], F32R, tag="wv")
                cos_sb = wpool.tile([128, NT, 64], F32R, tag="cos")
                sin_sb = wpool.tile([128, NT, 64], F32R, tag="sin")
                nc.sync.dma_start(cos_sb[:], cos_d.ap().rearrange("(t p) c -> p t c", p=128))
                nc.sync.dma_start(sin_sb[:], sin_d.ap().rearrange("(t p) c -> p t c", p=128))
                wqT_r = wqT.ap().rearrange("(kt p) f -> p kt f", p=128)
                wkT_r = wkT.ap().rearrange("(kt p) f -> p kt f", p=128)
                wvT_r = wvT.ap().rearrange("(kt p) f -> p kt f", p=128)
                # interleave x-tile prefetches into the weight stream so the
                # PE can chase the arriving weights through tb=0..2
                xq_tiles = {}
                for tb in range(3):
                    xq_tiles[tb] = xpool.tile([128, NT, 128], F32R, tag="xq", name=f"xq{tb}")
                nc.sync.dma_start(xq_tiles[0][:], xT_r[:, :, ts(0, 128)])
                for kt in range(NT):
                    nc.sync.dma_start(wq_sb[:, kt, :], wqT_r[:, kt, :])
                    nc.sync.dma_start(wk_sb[:, kt, :], wkT_r[:, kt, :])
                    nc.sync.dma_start(wv_sb[:, kt, :], wvT_r[:, kt, :])
                    if kt in (2, 5):
                        tb = 1 if kt == 2 else 2
                        nc.sync.dma_start(xq_tiles[tb][:], xT_r[:, :, ts(tb, 128)])

                for tb in range(NT):
                    if tb in xq_tiles:
                        xq = xq_tiles[tb]
                    else:
                        xq = xpool.tile([128, NT, 128], F32R, tag="xq")
                        nc.sync.dma_start(xq[:], xT_r[:, :, ts(tb, 128)])
                    pq = ps512.tile([128, 512], F32, tag="ps512")
                    pk = ps512.tile([128, 512], F32, tag="ps512")
                    pv = ps512.tile([128, 512], F32, tag="ps512")
                    for kt in range(NT):
                        f = dict(start=(kt == 0), stop=(kt == NT - 1))
                        nc.tensor.matmul(pq[:], xq[:, kt, :], wq_sb[:, kt, :], **f)
                        nc.tensor.matmul(pk[:], xq[:, kt, :], wk_sb[:, kt, :], **f)
                        nc.tensor.matmul(pv[:], xq[:, kt, :], wv_sb[:, kt, :], **f)
                    nc.vector.tensor_copy(vh_cb[tb // 4][:, tb % 4, :], pv[:])

                    # RoPE with broadcast APs: 4 wide DVE ops per tensor.
                    cos_b = cos_sb[:, tb, :].unsqueeze(1).unsqueeze(1) \
                        .broadcast_to((128, HG, 2, 64))
                    sin_b = sin_sb[:, tb, :].unsqueeze(1).broadcast_to((128, HG, 64))
                    for (ps, spill_r, rtag, stag) in (
                        (pq, q_spill_r, "qrot", "stq"),
                        (pk, k_spill_r, "krot", "stk"),
                    ):
                        ps_r = ps[:].rearrange("p (hb half j) -> p hb half j",
                                               hb=HG, half=2, j=64)
                        rot = rot_pool.tile([128, 512], F32R, tag=rtag)
                        rot_r = rot[:].rearrange("p (hb half j) -> p hb half j",
                                                 hb=HG, half=2, j=64)
                        tmp = tmp_pool.tile([128, HG, 2, 64], F32R, tag="tmp")
                        # tmp_lo = -q_hi * sin ; tmp_hi = +q_lo * sin
                        nc.vector.scalar_tensor_tensor(
                            tmp[:, :, 0, :], ps_r[:, :, 1, :], -1.0, sin_b,
                            op0=mybir.AluOpType.mult, op1=mybir.AluOpType.mult)
                        nc.vector.tensor_mul(tmp[:, :, 1, :], ps_r[:, :, 0, :], sin_b)
                        # rot = q * cos + tmp
                        nc.vector.tensor_mul(rot_r[:], ps_r[:], cos_b)
                        nc.vector.tensor_add(rot[:], rot[:],
                                             tmp[:].rearrange("p a b c -> p (a b c)"))
                        st = stage.tile([128, HG, 128], F32R, tag=stag)
                        for hb in range(HG):
                            ptr = ps_sm.tile([128, 128], F32R, tag="small")
                            nc.tensor.transpose(ptr[:], rot[:, ts(hb, 128)], ident[:])
                            nc.scalar.copy(st[:, hb, :], ptr[:])
                        nc.sync.dma_start(spill_r[tb // 4][:, :, ts(tb % 4, 128)], st[:])

            # ---------- Phase B+C: attention + output projection -----------
            with ExitStack() as bctx:
                mpool = bctx.enter_context(tc.tile_pool(name="mpool", bufs=1))
                pt_pool = bctx.enter_context(tc.tile_pool(name="pt", bufs=8))
                nrm = bctx.enter_context(tc.tile_pool(name="nrm", bufs=2))
                att_pool = bctx.enter_context(tc.tile_pool(name="attp", bufs=1))
                ystage = bctx.enter_context(tc.tile_pool(name="ystage", bufs=4))

                msk_sb = mpool.tile([128, 4, 512], F32R)
                nc.sync.dma_start(msk_sb[:], msk_d.ap().rearrange("o p q -> p o q"))
                wo_sb = mpool.tile([128, G, D], F32R, tag="wo")
                nc.sync.dma_start(wo_sb[:], woT.ap().rearrange("(ft p) d -> p ft d", p=128))
                qh_cb, kh_cb = [], []
                for cb in range(4):
                    qh = mpool.tile([128, HG, 512], F32R, tag=f"qh{cb}", name=f"qh{cb}")
                    kh = mpool.tile([128, HG, 512], F32R, tag=f"kh{cb}", name=f"kh{cb}")
                    for h in range(HG):
                        nc.gpsimd.dma_start(qh[:, h, :], q_spill_r[cb][:, h, :])
                        nc.gpsimd.dma_start(kh[:, h, :], k_spill_r[cb][:, h, :])
                    qh_cb.append(qh)
                    kh_cb.append(kh)
                attn_sb = att_pool.tile([128, HG, S], F32R, tag="attn_sb")

                for qb in range(NQB):
                    nkt = 4 * qb + 4
                    for h in range(HG):
                        p_att = ps512.tile([128, 512], F32, tag="ps512")
                        p_den = ps_sm.tile([1, 512], F32, tag="small")
                        for kt in range(nkt):
                            p_s = ps512.tile([128, 512], F32, tag="ps512")
                            nc.tensor.matmul(p_s[:],
                                             kh_cb[kt // 4][:, h, ts(kt % 4, 128)],
                                             qh_cb[qb][:, h, :],
                                             start=True, stop=True)
                            pt = pt_pool.tile([128, 512], F32R, tag="pt")
                            nc.scalar.activation(pt[:], p_s[:],
                                                 mybir.ActivationFunctionType.Exp,
                                                 scale=1.0 / math.sqrt(DH))
                            if kt >= 4 * qb:
                                nc.vector.tensor_mul(pt[:], pt[:],
                                                     msk_sb[:, kt - 4 * qb, :])
                            f = dict(start=(kt == 0), stop=(kt == nkt - 1))
                            nc.tensor.matmul(p_att[:],
                                             vh_cb[kt // 4][:, kt % 4, ts(h, 128)],
                                             pt[:], **f)
                            nc.tensor.matmul(p_den[:], ones[:], pt[:], **f)
                        recip = nrm.tile([1, 512], F32, tag="recip")
                        nc.vector.reciprocal_approx_fast(recip[:], p_den[:])
                        rb = nrm.tile([128, 512], F32, tag="rb")
                        nc.gpsimd.partition_broadcast(rb[:], recip[:])
                        nc.vector.tensor_mul(attn_sb[:, h, ts(qb, 512)],
                                             p_att[:], rb[:])
                    # output projection for this query block
                    for qt in range(4 * qb, 4 * qb + 4):
                        for ddb in range(NQB):
                            py = ps512.tile([128, 512], F32, tag="ps512")
                            for ft in range(G):
                                nc.tensor.matmul(py[:], attn_sb[:, ft, ts(qt, 128)],
                                                 wo_sb[:, ft, ts(ddb, 512)],
                                                 start=(ft == 0), stop=(ft == G - 1))
                            y_sb = ystage.tile([128, 512], F32, tag="ysb")
                            nc.scalar.copy(y_sb[:], py[:])
                            nc.sync.dma_start(y.ap()[ts(qt, 128), ts(ddb, 512)],
                                              y_sb[:])

    nc.compile()
    _cache[key] = nc
    return nc


def _in_maps(hidden_q, Wq, Wk, Wv, Wo):
    xs = (hidden_q.astype(np.float32) / math.sqrt(D))
    xT = [np.ascontiguousarray(xs[b].T) for b in range(B)]     # [D, S] each
    cos_t, sin_t = _rope_tables()
    masks = _mask_tiles()
    wo_s = Wo.astype(np.float32) / math.sqrt(H * DH)
    in_maps = []
    for c in range(8):
        b, g = c // G, c % G
        rows = slice(F * g, F * (g + 1))
        in_maps.append({
            "xT": xT[b],
            "wqT": np.ascontiguousarray(Wq[rows, :].T),
            "wkT": np.ascontiguousarray(Wk[rows, :].T),
            "wvT": np.ascontiguousarray(Wv[rows, :].T),
            "woT": np.ascontiguousarray(wo_s[:, rows].T),
            "cos": cos_t, "sin": sin_t, "masks": masks,
        })
    return in_maps


def kernel(hidden_q, attention_mask, position_bias, Wq, Wk, Wv, Wo):
    hidden_q = np.asarray(hidden_q)
    Wq, Wk, Wv, Wo = (np.asarray(w) for w in (Wq, Wk, Wv, Wo))
    assert hidden_q.shape == (B, S, D)
    in_maps = _in_maps(hidden_q, Wq, Wk, Wv, Wo)
    nc = _build()
    res = run_bass_kernel_spmd(nc, in_maps, core_ids=list(range(8)))
    _cache["last_results"] = res
    out = np.zeros((B, S, D), np.float32)
    for c in range(8):
        out[c // G] += res.results[c]["y"]
    return out



# revision 4
# speedup vs baseline: 1.3396x; 1.3396x over previous
"""Multi-head causal self-attention with RoPE on 8 Trainium2 NeuronCores.

Reference computation (B=2, S=2048, D=2048, H=16, DH=128):
    xs = hidden_q / sqrt(D)
    q,k,v = xs @ {Wq,Wk,Wv}.T        (reshaped to [B,H,S,DH])
    q,k <- RoPE(q,k)
    scores = q @ k.T / sqrt(DH)  (causal masked)
    p = softmax(scores); attn = p @ v
    out = (attn / sqrt(H*DH)) @ Wo.T

Sharding: 8 cores = 2 (batch) x 4 (head-groups of 4 heads).  Each core
computes its head-group's projections, attention and a partial output
projection; the host sums the 4 partials per batch.

All matmul inputs are bf16 (1 cyc/row on the PE, same as f32r, but
halves DMA/SBUF and enables DVE 2-byte fast modes + DMA-engine
transposes).  Softmax denominators are accumulated on the DVE and
summed across partitions by a gpsimd partition_all_reduce - no PE
denominator matmuls.  Q/K tiles are transposed by the DMA engines
(XBAR), not the PE.  Diagonal (causal-edge) blocks are computed on
sliced query ranges.  Projection / output-projection matmuls are
interleaved into the attention pipeline as PE filler so the tensor
engine queue never drains (keeps the PE p-state at max clock).
"""

import math
from collections import deque
from contextlib import ExitStack

import numpy as np
import ml_dtypes

import concourse.bass as bass
import concourse.bass_isa as bass_isa
import concourse.mybir as mybir
import concourse.tile as tile
from concourse import bacc
from concourse.bass import ts
from concourse.bass_utils import run_bass_kernel_spmd

B, S, D, H, DH = 2, 2048, 2048, 16, 128
BASE = 10000.0
G = 4              # head-groups (cores per batch)
HG = H // G        # heads per group = 4
F = HG * DH        # features per group = 512
NT = S // 128      # 16 token tiles
NQB = S // 512     # 4 query blocks
F32 = mybir.dt.float32
BF16 = mybir.dt.bfloat16
SCALE = 1.0 / math.sqrt(DH)

_cache = {}


def _rope_tables():
    inv_freq = 1.0 / (BASE ** (np.arange(0, DH, 2, dtype=np.float64) / DH))
    t = np.arange(S, dtype=np.float64)
    freqs = np.outer(t, inv_freq)                       # [S, 64]
    return (np.cos(freqs).astype(ml_dtypes.bfloat16),
            np.sin(freqs).astype(ml_dtypes.bfloat16))


def _build(reps=1):
    key = ("nc", reps)
    if key in _cache:
        return _cache[key]
    nc = bacc.Bacc("TRN2", target_bir_lowering=False, debug=False, num_devices=8)

    xT = nc.dram_tensor("xT", [D, S], BF16, kind="ExternalInput")
    wqT = nc.dram_tensor("wqT", [D, F], BF16, kind="ExternalInput")
    wkT = nc.dram_tensor("wkT", [D, F], BF16, kind="ExternalInput")
    wvT = nc.dram_tensor("wvT", [D, F], BF16, kind="ExternalInput")
    woT = nc.dram_tensor("woT", [F, D], BF16, kind="ExternalInput")
    cos_d = nc.dram_tensor("cos", [S, 64], BF16, kind="ExternalInput")
    sin_d = nc.dram_tensor("sin", [S, 64], BF16, kind="ExternalInput")
    tri_d = nc.dram_tensor("tri", [128, 128], BF16, kind="ExternalInput")
    y = nc.dram_tensor("y", [S, D], F32, kind="ExternalOutput")

    xT_r = xT.ap().rearrange("(kt p) s -> p kt s", p=128)       # [128, 16, S]
    wqT_r = wqT.ap().rearrange("(kt p) f -> p kt f", p=128)
    wkT_r = wkT.ap().rearrange("(kt p) f -> p kt f", p=128)
    wvT_r = wvT.ap().rearrange("(kt p) f -> p kt f", p=128)

    with tile.TileContext(nc) as tc, ExitStack() as ctx:
        pers = ctx.enter_context(tc.tile_pool(name="pers", bufs=1))
        xpool = ctx.enter_context(tc.tile_pool(name="xpool", bufs=2))
        stage = ctx.enter_context(tc.tile_pool(name="stage", bufs=3))
        tmp2 = ctx.enter_context(tc.tile_pool(name="tmp2", bufs=2))
        ptp = ctx.enter_context(tc.tile_pool(name="ptp", bufs=4))
        yst = ctx.enter_context(tc.tile_pool(name="yst", bufs=3))
        ps_pp = ctx.enter_context(tc.tile_pool(name="ps_pp", bufs=2, space="PSUM"))
        ps_sc = ctx.enter_context(tc.tile_pool(name="ps_sc", bufs=2, space="PSUM"))
        ps_at = ctx.enter_context(tc.tile_pool(name="ps_at", bufs=2, space="PSUM"))

        # ---------------- persistent SBUF ----------------
        wq_sb = pers.tile([128, NT, F], BF16, tag="wq")
        wk_sb = pers.tile([128, NT, F], BF16, tag="wk")
        wv_sb = pers.tile([128, NT, F], BF16, tag="wv")
        wo_sb = pers.tile([128, G, D], BF16, tag="wo")
        cos_sb = pers.tile([128, NT, 64], BF16, tag="cos")
        sin_sb = pers.tile([128, NT, 64], BF16, tag="sin")
        tri_sb = pers.tile([128, 128], BF16, tag="tri")
        qT_sb = pers.tile([128, HG, S], BF16, tag="qT")
        kT_sb = pers.tile([128, HG, S], BF16, tag="kT")
        v_sb = pers.tile([128, NT, F], BF16, tag="v")
        attn_sb = pers.tile([128, HG, S], BF16, tag="attn")

        nc.sync.dma_start(cos_sb[:], cos_d.ap().rearrange("(t p) c -> p t c", p=128))
        nc.sync.dma_start(sin_sb[:], sin_d.ap().rearrange("(t p) c -> p t c", p=128))
        nc.sync.dma_start(tri_sb[:], tri_d.ap())
        nc.sync.dma_start(wo_sb[:], woT.ap().rearrange("(ft p) d -> p ft d", p=128))
        # weights per-kt in q,k,v order (matches serialized projection order)
        for w_sb, w_r in ((wq_sb, wqT_r), (wk_sb, wkT_r), (wv_sb, wvT_r)):
            for kt in range(NT):
                nc.sync.dma_start(w_sb[:, kt, :], w_r[:, kt, :])

        x_tiles = {}

        def fetch_x(sblk):
            xt = xpool.tile([128, NT, 512], BF16, tag="x")
            nc.sync.dma_start(xt[:], xT_r[:, :, ts(sblk, 512)])
            x_tiles[sblk] = xt

        fetch_x(0)
        fetch_x(1)

        # ---------------- filler machinery ----------------
        filler = deque()

        def pull(n):
            for _ in range(n):
                if filler:
                    filler.popleft()()

        def drain():
            while filler:
                filler.popleft()()

        # ---------------- projection of one token tile ----------------
        def add_proj_units(tb):
            """24 filler units (2 matmuls each): serialized q, k, v
            projections of token tile tb with rope/transpose chase ops."""
            sblk = tb // 4
            xq = x_tiles[sblk]
            st = {}

            def mk_unit(w_sb, kt0, name, first, last):
                def unit():
                    if first:
                        st[name] = ps_pp.tile([128, 512], F32, tag="pp", name="pp")
                    ps = st[name]
                    for kk in (kt0, kt0 + 1):
                        nc.tensor.matmul(
                            ps[:], xq[:, kk, ts(tb % 4, 128)], w_sb[:, kk, :],
                            start=(kk == 0), stop=(kk == NT - 1))
                    if last:
                        finish(name, ps)
                return unit

            def finish(name, ps):
                if name == "v":
                    nc.vector.tensor_copy(v_sb[:, tb, :], ps[:])
                    return
                dst = qT_sb if name == "q" else kT_sb
                sb = stage.tile([128, 512], BF16, tag="qsb")
                nc.scalar.copy(sb[:], ps[:])
                q4 = sb[:].rearrange("p (hb half j) -> p hb half j", hb=HG,
                                     half=2, j=64)
                cos_b = cos_sb[:, tb, :].unsqueeze(1).unsqueeze(1) \
                    .broadcast_to((128, HG, 2, 64))
                sin_b = sin_sb[:, tb, :].unsqueeze(1).broadcast_to((128, HG, 64))
                tmp = tmp2.tile([128, HG, 2, 64], BF16, tag="rtmp")
                nc.vector.scalar_tensor_tensor(
                    tmp[:, :, 0, :], q4[:, :, 1, :], -1.0, sin_b,
                    op0=mybir.AluOpType.mult, op1=mybir.AluOpType.mult)
                nc.vector.tensor_mul(tmp[:, :, 1, :], q4[:, :, 0, :], sin_b)
                rot = stage.tile([128, 512], BF16, tag="rot")
                rot4 = rot[:].rearrange("p (hb half j) -> p hb half j", hb=HG,
                                        half=2, j=64)
                nc.vector.tensor_mul(rot4[:], q4[:], cos_b)
                nc.vector.tensor_add(rot[:], rot[:],
                                     tmp[:].rearrange("p a b c -> p (a b c)"))
                for hb in range(HG):
                    nc.sync.dma_start_transpose(dst[:, hb, ts(tb, 128)],
                                                rot[:, ts(hb, 128)])

            for name, w_sb in (("q", wq_sb), ("k", wk_sb), ("v", wv_sb)):
                for u in range(8):
                    filler.append(mk_unit(w_sb, 2 * u, name, u == 0, u == 7))

        # ---------------- output projection units ----------------
        def add_outproj_units(qb):
            """32 filler units (2 matmuls each): y partial for query block
            qb; each (qt, ddb) chunk is two units + copy/DMA chase."""
            st = {}

            def mk_unit(qt, ddb, first, last):
                def unit():
                    if first:
                        st["py"] = ps_pp.tile([128, 512], F32, tag="pp", name="pp")
                    py = st["py"]
                    fts = (0, 1) if first else (2, 3)
                    for ft in fts:
                        nc.tensor.matmul(py[:], attn_sb[:, ft, ts(qt, 128)],
                                         wo_sb[:, ft, ts(ddb, 512)],
                                         start=(ft == 0), stop=(ft == G - 1))
                    if last:
                        y_sb = yst.tile([128, 512], F32, tag="ysb")
                        if (qt + ddb) % 2 == 0:
                            nc.scalar.copy(y_sb[:], py[:])
                        else:
                            nc.vector.tensor_copy(y_sb[:], py[:])
                        nc.sync.dma_start(y.ap()[ts(qt, 128), ts(ddb, 512)],
                                          y_sb[:])
                return unit

            for qt in range(4 * qb, 4 * qb + 4):
                for ddb in range(NQB):
                    filler.append(mk_unit(qt, ddb, True, False))
                    filler.append(mk_unit(qt, ddb, False, True))

        # ---------------- attention for one (qb, h) ----------------
        def attn_h(qb, h):
            nkt = 4 * qb + 4
            pairs = nkt // 2
            p_att = ps_at.tile([128, 512], F32, tag="att")
            den = tmp2.tile([128, 512], F32, tag="den")
            pend = None           # (pt, q0s, hfs) awaiting attnV emission

            def emit_attnv(p):
                pt, q0s, kts = p
                for (kt, q0, hf) in kts:
                    nc.tensor.matmul(p_att[:, q0:512],
                                     v_sb[:, kt, ts(h, 128)],
                                     pt[:, hf, q0:512],
                                     start=(kt == 0), stop=(kt == nkt - 1))

            for i in range(pairs):
                kt0, kt1 = 2 * i, 2 * i + 1
                j0, j1 = kt0 - 4 * qb, kt1 - 4 * qb          # >=0 on diagonal
                q00 = 128 * j0 if j0 >= 0 else 0
                q01 = 128 * j1 if j1 >= 0 else 0
                # scores pair -> one 2-bank psum tile
                psc = ps_sc.tile([128, 2, 512], F32, tag="sc")
                nc.tensor.matmul(psc[:, 0, q00:512],
                                 kT_sb[:, h, ts(kt0, 128)],
                                 qT_sb[:, h, 512 * qb + q00:512 * (qb + 1)],
                                 start=True, stop=True)
                nc.tensor.matmul(psc[:, 1, q01:512],
                                 kT_sb[:, h, ts(kt1, 128)],
                                 qT_sb[:, h, 512 * qb + q01:512 * (qb + 1)],
                                 start=True, stop=True)
                # exp of the pair (sliced to the union of valid cols)
                pt = ptp.tile([128, 2, 512], BF16, tag="pt")
                nc.scalar.activation(pt[:, :, q00:512], psc[:, :, q00:512],
                                     mybir.ActivationFunctionType.Exp,
                                     scale=SCALE)
                # causal masks on diagonal blocks
                if j0 >= 0:
                    nc.vector.tensor_mul(pt[:, 0, ts(j0, 128)],
                                         pt[:, 0, ts(j0, 128)], tri_sb[:])
                if j1 >= 0:
                    nc.vector.tensor_mul(pt[:, 1, ts(j1, 128)],
                                         pt[:, 1, ts(j1, 128)], tri_sb[:])
                # denominator accumulation (DVE)
                if j0 >= 0:
                    if i == 0:
                        nc.vector.tensor_copy(den[:], pt[:, 0, :])
                    else:
                        nc.vector.tensor_add(den[:, q00:], den[:, q00:],
                                             pt[:, 0, q00:])
                    nc.vector.tensor_add(den[:, q01:], den[:, q01:],
                                         pt[:, 1, q01:])
                else:
                    dt_ = tmp2.tile([128, 512], BF16, tag="dtmp")
                    nc.vector.tensor_add(dt_[:], pt[:, 0, :], pt[:, 1, :])
                    if i == 0:
                        nc.vector.tensor_copy(den[:], dt_[:])
                    else:
                        nc.vector.tensor_add(den[:], den[:], dt_[:])
                pull(3)
                if pend is not None:
                    emit_attnv(pend)
                pend = (pt, (q00, q01),
                        ((kt0, q00, 0), (kt1, q01, 1)))
            pull(2)
            emit_attnv(pend)
            # normalize: cross-partition sum, reciprocal, scale
            rb = tmp2.tile([128, 512], F32, tag="rb")
            nc.gpsimd.partition_all_reduce(rb[:], den[:], 128,
                                           bass_isa.ReduceOp.add)
            rcp = tmp2.tile([128, 512], F32, tag="rcp")
            nc.vector.reciprocal_approx_fast(rcp[:], rb[:])
            nc.vector.tensor_mul(attn_sb[:, h, ts(qb, 512)], p_att[:], rcp[:])

        # ---------------- schedule ----------------
        for tb in range(4):                     # projections for sblk 0
            add_proj_units(tb)
        drain()
        for qb in range(NQB):
            if qb < 3:
                if qb >= 1:
                    fetch_x(qb + 1)
                for tb in range(4 * (qb + 1), 4 * (qb + 1) + 4):
                    add_proj_units(tb)
            else:
                add_outproj_units(1)
                add_outproj_units(2)
            for h in range(HG):
                attn_h(qb, h)
            drain()
            if qb == 0:
                add_outproj_units(0)
        add_outproj_units(3)
        drain()

    nc.compile()
    _cache[key] = nc
    return nc


def _in_maps(hidden_q, Wq, Wk, Wv, Wo):
    bf = ml_dtypes.bfloat16
    xs = (np.asarray(hidden_q, np.float32) / math.sqrt(D))
    xT = [np.ascontiguousarray(xs[b].T).astype(bf) for b in range(B)]
    cos_t, sin_t = _rope_tables()
    tri = np.tril(np.ones((128, 128), np.float32)).T.astype(bf)  # [k,q] q>=k
    wo_s = np.asarray(Wo, np.float32) / math.sqrt(H * DH)
    in_maps = []
    for c in range(8):
        b, g = c // G, c % G
        rows = slice(F * g, F * (g + 1))
        in_maps.append({
            "xT": xT[b],
            "wqT": np.ascontiguousarray(np.asarray(Wq, np.float32)[rows, :].T).astype(bf),
            "wkT": np.ascontiguousarray(np.asarray(Wk, np.float32)[rows, :].T).astype(bf),
            "wvT": np.ascontiguousarray(np.asarray(Wv, np.float32)[rows, :].T).astype(bf),
            "woT": np.ascontiguousarray(wo_s[:, rows].T).astype(bf),
            "cos": cos_t, "sin": sin_t, "tri": tri,
        })
    return in_maps


def kernel(hidden_q, attention_mask, position_bias, Wq, Wk, Wv, Wo):
    hidden_q = np.asarray(hidden_q)
    assert hidden_q.shape == (B, S, D)
    in_maps = _in_maps(hidden_q, Wq, Wk, Wv, Wo)
    nc = _build()
    res = run_bass_kernel_spmd(nc, in_maps, core_ids=list(range(8)))
    _cache["last_results"] = res
    out = np.zeros((B, S, D), np.float32)
    for c in range(8):
        out[c // G] += res.results[c]["y"]
    return out


# revision 6
# speedup vs baseline: 1.4060x; 1.0496x over previous
"""Multi-head causal self-attention with RoPE on 8 Trainium2 NeuronCores.

Reference computation (B=2, S=2048, D=2048, H=16, DH=128):
    xs = hidden_q / sqrt(D)
    q,k,v = xs @ {Wq,Wk,Wv}.T        (reshaped to [B,H,S,DH])
    q,k <- RoPE(q,k)
    scores = q @ k.T / sqrt(DH)  (causal masked)
    p = softmax(scores); attn = p @ v
    out = (attn / sqrt(H*DH)) @ Wo.T

Sharding: 8 cores = 2 (batch) x 4 (head-groups of 4 heads).  Each core
computes its head-group's projections, attention and a partial output
projection; the host sums the 4 partials per batch.

All matmul inputs are bf16 (1 cyc/row on the PE, same as f32r, but
halves DMA/SBUF and enables DVE 2-byte fast modes + DMA-engine
transposes).  Softmax denominators are accumulated on the DVE and
summed across partitions by a gpsimd partition_all_reduce - no PE
denominator matmuls.  Q/K tiles are transposed by the DMA engines
(XBAR), not the PE.  Diagonal (causal-edge) blocks are computed on
sliced query ranges.  Projection / output-projection matmuls are
interleaved into the attention pipeline as PE filler so the tensor
engine queue never drains (keeps the PE p-state at max clock).
"""

import math
from collections import deque
from contextlib import ExitStack

import numpy as np
import ml_dtypes

import concourse.bass as bass
import concourse.bass_isa as bass_isa
import concourse.mybir as mybir
import concourse.tile as tile
from concourse import bacc
from concourse.bass import ts
from concourse.bass_utils import run_bass_kernel_spmd

B, S, D, H, DH = 2, 2048, 2048, 16, 128
BASE = 10000.0
G = 4              # head-groups (cores per batch)
HG = H // G        # heads per group = 4
F = HG * DH        # features per group = 512
NT = S // 128      # 16 token tiles
NQB = S // 512     # 4 query blocks
F32 = mybir.dt.float32
BF16 = mybir.dt.bfloat16
SCALE = 1.0 / math.sqrt(DH)

_cache = {}


def _rope_tables():
    inv_freq = 1.0 / (BASE ** (np.arange(0, DH, 2, dtype=np.float64) / DH))
    t = np.arange(S, dtype=np.float64)
    freqs = np.outer(t, inv_freq)                       # [S, 64]
    return (np.cos(freqs).astype(ml_dtypes.bfloat16),
            np.sin(freqs).astype(ml_dtypes.bfloat16))


def _build(reps=1):
    key = ("nc", reps)
    if key in _cache:
        return _cache[key]
    nc = bacc.Bacc("TRN2", target_bir_lowering=False, debug=False, num_devices=8)

    xT = nc.dram_tensor("xT", [D, S], BF16, kind="ExternalInput")
    wqT = nc.dram_tensor("wqT", [D, F], BF16, kind="ExternalInput")
    wkT = nc.dram_tensor("wkT", [D, F], BF16, kind="ExternalInput")
    wvT = nc.dram_tensor("wvT", [D, F], BF16, kind="ExternalInput")
    woT = nc.dram_tensor("woT", [F, D], BF16, kind="ExternalInput")
    cos_d = nc.dram_tensor("cos", [S, 64], BF16, kind="ExternalInput")
    sin_d = nc.dram_tensor("sin", [S, 64], BF16, kind="ExternalInput")
    tri_d = nc.dram_tensor("tri", [128, 128], BF16, kind="ExternalInput")
    y = nc.dram_tensor("y", [S, D], F32, kind="ExternalOutput")

    xT_r = xT.ap().rearrange("(kt p) s -> p kt s", p=128)       # [128, 16, S]
    wqT_r = wqT.ap().rearrange("(kt p) f -> p kt f", p=128)
    wkT_r = wkT.ap().rearrange("(kt p) f -> p kt f", p=128)
    wvT_r = wvT.ap().rearrange("(kt p) f -> p kt f", p=128)

    with tile.TileContext(nc) as tc, ExitStack() as ctx:
        pers = ctx.enter_context(tc.tile_pool(name="pers", bufs=1))
        xpool = ctx.enter_context(tc.tile_pool(name="xpool", bufs=2))
        stage = ctx.enter_context(tc.tile_pool(name="stage", bufs=3))
        tmp2 = ctx.enter_context(tc.tile_pool(name="tmp2", bufs=2))
        ptp = ctx.enter_context(tc.tile_pool(name="ptp", bufs=5))
        yst = ctx.enter_context(tc.tile_pool(name="yst", bufs=3))
        ps_pp = ctx.enter_context(tc.tile_pool(name="ps_pp", bufs=3, space="PSUM"))
        ps_sc = ctx.enter_context(tc.tile_pool(name="ps_sc", bufs=3, space="PSUM"))
        ps_at = ctx.enter_context(tc.tile_pool(name="ps_at", bufs=2, space="PSUM"))

        # ---------------- persistent SBUF ----------------
        wq_sb = pers.tile([128, NT, F], BF16, tag="wq")
        wk_sb = pers.tile([128, NT, F], BF16, tag="wk")
        wv_sb = pers.tile([128, NT, F], BF16, tag="wv")
        wo_sb = pers.tile([128, G, D], BF16, tag="wo")
        cos_sb = pers.tile([128, NT, 64], BF16, tag="cos")
        sin_sb = pers.tile([128, NT, 64], BF16, tag="sin")
        tri_sb = pers.tile([128, 128], BF16, tag="tri")
        qT_sb = pers.tile([128, HG, S], BF16, tag="qT")
        kT_sb = pers.tile([128, HG, S], BF16, tag="kT")
        v_sb = pers.tile([128, NT, F], BF16, tag="v")
        attn_sb = pers.tile([128, HG, S], BF16, tag="attn")

        x_tiles = {}

        def fetch_x(sblk, chunks=1):
            xt = xpool.tile([128, NT, 512], BF16, tag="x", name="x")
            if chunks == 1:
                nc.sync.dma_start(xt[:], xT_r[:, :, ts(sblk, 512)])
            x_tiles[sblk] = xt
            return xt

        # startup order matches first-consumer order: x0 chunks interleaved
        # with wq tiles, then cos/sin/tri, wk, wv, x1, wo (needed last)
        x0 = fetch_x(0, chunks=4)
        for c in range(4):
            nc.sync.dma_start(x0[:, ts(c, 4), :],
                              xT_r[:, ts(c, 4), ts(0, 512)])
            for kt in range(4 * c, 4 * c + 4):
                nc.sync.dma_start(wq_sb[:, kt, :], wqT_r[:, kt, :])
        nc.sync.dma_start(cos_sb[:], cos_d.ap().rearrange("(t p) c -> p t c", p=128))
        nc.sync.dma_start(sin_sb[:], sin_d.ap().rearrange("(t p) c -> p t c", p=128))
        nc.sync.dma_start(tri_sb[:], tri_d.ap())
        for w_sb, w_r in ((wk_sb, wkT_r), (wv_sb, wvT_r)):
            for kt in range(NT):
                nc.sync.dma_start(w_sb[:, kt, :], w_r[:, kt, :])
        fetch_x(1)
        nc.sync.dma_start(wo_sb[:], woT.ap().rearrange("(ft p) d -> p ft d", p=128))

        # ---------------- filler machinery ----------------
        filler = deque()

        def pull(n):
            for _ in range(n):
                if filler:
                    filler.popleft()()

        def drain():
            while filler:
                filler.popleft()()

        # ---------------- projection of one token tile ----------------
        def add_proj_units(tb):
            """24 filler units (2 matmuls each): serialized q, k, v
            projections of token tile tb with rope/transpose chase ops."""
            sblk = tb // 4
            xq = x_tiles[sblk]
            st = {}

            def mk_unit(w_sb, kt0, name, first, last):
                def unit():
                    if first:
                        st[name] = ps_pp.tile([128, 512], F32, tag="pp", name="pp")
                    ps = st[name]
                    for kk in (kt0, kt0 + 1):
                        nc.tensor.matmul(
                            ps[:], xq[:, kk, ts(tb % 4, 128)], w_sb[:, kk, :],
                            start=(kk == 0), stop=(kk == NT - 1))
                    if last:
                        finish(name, ps)
                return unit

            def finish(name, ps):
                if name == "v":
                    nc.vector.tensor_copy(v_sb[:, tb, :], ps[:])
                    return
                dst = qT_sb if name == "q" else kT_sb
                sb = stage.tile([128, 512], BF16, tag="qsb")
                nc.scalar.copy(sb[:], ps[:])
                q4 = sb[:].rearrange("p (hb half j) -> p hb half j", hb=HG,
                                     half=2, j=64)
                cos_b = cos_sb[:, tb, :].unsqueeze(1).unsqueeze(1) \
                    .broadcast_to((128, HG, 2, 64))
                sin_b = sin_sb[:, tb, :].unsqueeze(1).broadcast_to((128, HG, 64))
                tmp = tmp2.tile([128, HG, 2, 64], BF16, tag="rtmp")
                nc.vector.scalar_tensor_tensor(
                    tmp[:, :, 0, :], q4[:, :, 1, :], -1.0, sin_b,
                    op0=mybir.AluOpType.mult, op1=mybir.AluOpType.mult)
                nc.vector.tensor_mul(tmp[:, :, 1, :], q4[:, :, 0, :], sin_b)
                rot = stage.tile([128, 512], BF16, tag="rot")
                rot4 = rot[:].rearrange("p (hb half j) -> p hb half j", hb=HG,
                                        half=2, j=64)
                nc.vector.tensor_mul(rot4[:], q4[:], cos_b)
                nc.vector.tensor_add(rot[:], rot[:],
                                     tmp[:].rearrange("p a b c -> p (a b c)"))
                for hb in range(HG):
                    nc.sync.dma_start_transpose(dst[:, hb, ts(tb, 128)],
                                                rot[:, ts(hb, 128)])

            for name, w_sb in (("q", wq_sb), ("k", wk_sb), ("v", wv_sb)):
                for u in range(8):
                    filler.append(mk_unit(w_sb, 2 * u, name, u == 0, u == 7))

        # ---------------- output projection units ----------------
        def add_outproj_units(qb):
            """32 filler units (2 matmuls each): y partial for query block
            qb; each (qt, ddb) chunk is two units + copy/DMA chase."""
            st = {}

            def mk_unit(qt, ddb, first, last):
                def unit():
                    if first:
                        st["py"] = ps_pp.tile([128, 512], F32, tag="pp", name="pp")
                    py = st["py"]
                    fts = (0, 1) if first else (2, 3)
                    for ft in fts:
                        nc.tensor.matmul(py[:], attn_sb[:, ft, ts(qt, 128)],
                                         wo_sb[:, ft, ts(ddb, 512)],
                                         start=(ft == 0), stop=(ft == G - 1))
                    if last:
                        y_sb = yst.tile([128, 512], F32, tag="ysb")
                        if (qt + ddb) % 2 == 0:
                            nc.scalar.copy(y_sb[:], py[:])
                        else:
                            nc.vector.tensor_copy(y_sb[:], py[:])
                        nc.sync.dma_start(y.ap()[ts(qt, 128), ts(ddb, 512)],
                                          y_sb[:])
                return unit

            for qt in range(4 * qb, 4 * qb + 4):
                for ddb in range(NQB):
                    filler.append(mk_unit(qt, ddb, True, False))
                    filler.append(mk_unit(qt, ddb, False, True))

        # ---------------- attention for one (qb, h) ----------------
        def attn_h(qb, h):
            nkt = 4 * qb + 4
            p_att = ps_at.tile([128, 512], F32, tag="att", name="att")
            den = tmp2.tile([128, 512], F32, tag="den", name="den")
            pts = {}
            t2 = {}

            def a_of(kt):
                q0 = max(0, 128 * (kt - 4 * qb))
                nc.tensor.matmul(p_att[:, q0:512], v_sb[:, kt, ts(h, 128)],
                                 pts[kt][:, q0:512],
                                 start=(kt == 0), stop=(kt == nkt - 1))

            for kt in range(nkt):
                j = kt - 4 * qb
                q0 = max(0, 128 * j)
                psc = ps_sc.tile([128, 512], F32, tag="sc", name="sc")
                nc.tensor.matmul(psc[:, q0:512],
                                 kT_sb[:, h, ts(kt, 128)],
                                 qT_sb[:, h, 512 * qb + q0:512 * (qb + 1)],
                                 start=True, stop=True)
                pt = ptp.tile([128, 512], BF16, tag="pt", name="pt")
                pts[kt] = pt
                nc.scalar.activation(pt[:, q0:512], psc[:, q0:512],
                                     mybir.ActivationFunctionType.Exp,
                                     scale=SCALE)
                if j >= 0:
                    # diagonal: triangle mask then sliced f32 add into den
                    nc.vector.tensor_mul(pt[:, ts(j, 128)], pt[:, ts(j, 128)],
                                         tri_sb[:])
                    if qb == 0 and kt == 0:
                        nc.vector.tensor_copy(den[:], pt[:])
                    else:
                        nc.vector.tensor_add(den[:, q0:], den[:, q0:],
                                             pt[:, q0:])
                else:
                    # off-diagonal: bf16 pair/quad tree, one f32 add per 4 kt
                    if kt % 2 == 1:
                        tt = tmp2.tile([128, 512], BF16, tag="t2", name="t2")
                        nc.vector.tensor_add(tt[:], pts[kt - 1][:], pt[:])
                        t2[kt // 2] = tt
                    if kt % 4 == 3:
                        t4 = tmp2.tile([128, 512], BF16, tag="t4", name="t4")
                        nc.vector.tensor_add(t4[:], t2[kt // 2 - 1][:],
                                             t2[kt // 2][:])
                        if kt == 3:
                            nc.vector.tensor_copy(den[:], t4[:])
                        else:
                            nc.vector.tensor_add(den[:], den[:], t4[:])
                pull(2 if kt % 2 == 0 else 1)
                if kt >= 2:
                    a_of(kt - 2)
            a_of(nkt - 2)
            a_of(nkt - 1)
            # normalize: cross-partition sum, reciprocal, scale
            rb = tmp2.tile([128, 512], F32, tag="rb")
            nc.gpsimd.partition_all_reduce(rb[:], den[:], 128,
                                           bass_isa.ReduceOp.add)
            rcp = tmp2.tile([128, 512], F32, tag="rcp")
            nc.vector.reciprocal_approx_fast(rcp[:], rb[:])
            nc.vector.tensor_mul(attn_sb[:, h, ts(qb, 512)], p_att[:], rcp[:])

        # ---------------- schedule ----------------
        for tb in range(4):                     # projections for sblk 0
            add_proj_units(tb)
        drain()
        for qb in range(NQB):
            if qb < 3:
                if qb < 2:
                    fetch_x(qb + 2)
                for tb in range(4 * (qb + 1), 4 * (qb + 1) + 4):
                    add_proj_units(tb)
            else:
                add_outproj_units(0)
                add_outproj_units(1)
                add_outproj_units(2)
            for h in range(HG):
                attn_h(qb, h)
            drain()
        add_outproj_units(3)
        drain()

    nc.compile()
    _cache[key] = nc
    return nc


def _in_maps(hidden_q, Wq, Wk, Wv, Wo):
    bf = ml_dtypes.bfloat16
    xs = (np.asarray(hidden_q, np.float32) / math.sqrt(D))
    xT = [np.ascontiguousarray(xs[b].T).astype(bf) for b in range(B)]
    cos_t, sin_t = _rope_tables()
    tri = np.tril(np.ones((128, 128), np.float32)).T.astype(bf)  # [k,q] q>=k
    wo_s = np.asarray(Wo, np.float32) / math.sqrt(H * DH)
    in_maps = []
    for c in range(8):
        b, g = c // G, c % G
        rows = slice(F * g, F * (g + 1))
        in_maps.append({
            "xT": xT[b],
            "wqT": np.ascontiguousarray(np.asarray(Wq, np.float32)[rows, :].T).astype(bf),
            "wkT": np.ascontiguousarray(np.asarray(Wk, np.float32)[rows, :].T).astype(bf),
            "wvT": np.ascontiguousarray(np.asarray(Wv, np.float32)[rows, :].T).astype(bf),
            "woT": np.ascontiguousarray(wo_s[:, rows].T).astype(bf),
            "cos": cos_t, "sin": sin_t, "tri": tri,
        })
    return in_maps


def kernel(hidden_q, attention_mask, position_bias, Wq, Wk, Wv, Wo):
    hidden_q = np.asarray(hidden_q)
    assert hidden_q.shape == (B, S, D)
    in_maps = _in_maps(hidden_q, Wq, Wk, Wv, Wo)
    nc = _build()
    res = run_bass_kernel_spmd(nc, in_maps, core_ids=list(range(8)))
    _cache["last_results"] = res
    out = np.zeros((B, S, D), np.float32)
    for c in range(8):
        out[c // G] += res.results[c]["y"]
    return out
